# revision 1
# baseline (speedup 1.0000x reference)
"""Trainium2 Bass kernel for nn_MetricLoss (pairwise metric loss, B=8192 D=128 k=4).

  d2[i,j] = sq_i + sq_j - 2*x_i.x_j
  loss_homo  = sum_{same group, i!=j} d2 / 24576
  loss_heter = sum_{g_i < g_j} relu(1 - d2) / 33538048

Circular half-window sharding over 8 NeuronCores: the 8192 rows form 64
blocks of 128.  Core p owns anchor blocks R = 8p..8p+7.  Every anchor
block processes column blocks (R+1 .. R+32) mod 64.  The host hands each
core a contiguous wrapped window xw = x^T[:, blocks 8p .. 8p+39] so the
device program is identical on all cores (pure SPMD): anchor tile t is
window block t, its columns are window blocks t+1..t+32.

Block-pair coverage: distance 1..31 exactly once, distance 32 twice (one
orientation recomputed bitwise-identically and subtracted on the host),
distance 0 (within-block cross-group pairs) via a per-anchor diagonal
pass, which also yields the homo loss via masked sums of the diagonal
Gram tiles (algebraic correction with sq recovered from the bias output).

Per (anchor tile t, 1024-column macro chunk):
  PSUM [128,1024] = G - sq_j/2  (2x fp32r matmul + 2x rank-1 ones x (-sq/2))
  pointwise relu((1-d2)/2) with fused per-slot column-sum accumulation:
    tiles 0-3 on ScalarE (activation Relu, bias=(1-sq_i)/2, accum_out),
    tiles 4-7 on VectorE (scalar_tensor_tensor add-then-max-with-0, accum_out)
"""
import sys

sys.path.insert(0, "/opt/trn_rl_repo")

import numpy as np
import concourse.bacc as bacc
import concourse.tile as tile
import concourse.mybir as mybir
from concourse import bass_utils
from contextlib import ExitStack

F32 = mybir.dt.float32
F32R = mybir.dt.float32r

B, D, K = 8192, 128, 4
NCORES = 8
RPC = B // NCORES          # rows per core (1024)
NT = RPC // 128            # anchor tiles per core (8)
WBLK = 32                  # column blocks per anchor tile
WINB = NT + WBLK           # window blocks: global blocks 8p .. 8p+39
WIN = WINB * 128           # 5120 columns
NMC = WBLK * 128 // 1024   # macro chunks per tile (4)
CNT_HOMO = float((B // K) * K * (K - 1))                 # 24576
CNT_HETER = float(K * K * (B // K) * (B // K - 1) // 2)  # 33538048

_CACHE = {}


def _build_program():
    nc = bacc.Bacc("TRN2", target_bir_lowering=False, debug=False)

    xw_in = nc.dram_tensor("xw_in", [128, WIN], F32R, kind="ExternalInput").ap()
    maskh_in = nc.dram_tensor("maskh_in", [128, 128], F32, kind="ExternalInput").ap()
    maskx_in = nc.dram_tensor("maskx_in", [128, 128], F32, kind="ExternalInput").ap()

    hacc_out = nc.dram_tensor("hacc_out", [128, NT * NMC], F32, kind="ExternalOutput").ap()
    s32_out = nc.dram_tensor("s32_out", [128, NT], F32, kind="ExternalOutput").ap()
    kacc_out = nc.dram_tensor("kacc_out", [128, NT], F32, kind="ExternalOutput").ap()
    macc_out = nc.dram_tensor("macc_out", [128, NT], F32, kind="ExternalOutput").ap()
    hb_out = nc.dram_tensor("hb_out", [128, NT], F32, kind="ExternalOutput").ap()

    Relu = mybir.ActivationFunctionType.Relu
    Copy = mybir.ActivationFunctionType.Copy
    ADD = mybir.AluOpType.add
    MULT = mybir.AluOpType.mult
    MAX = mybir.AluOpType.max

    NW512 = WIN // 512          # 10 exact 512-chunks
    ACT_TILES = (0, 1, 2, 4, 6)   # pointwise on ScalarE; rest on VectorE

    with tile.TileContext(nc) as tc, ExitStack() as ctx:
        cp = ctx.enter_context(tc.tile_pool(name="cp", bufs=1))
        dp = ctx.enter_context(tc.tile_pool(name="dp", bufs=1, space="DRAM"))
        wp = ctx.enter_context(tc.tile_pool(name="wp", bufs=3))
        rp = ctx.enter_context(tc.tile_pool(name="rp", bufs=4))
        r2p = ctx.enter_context(tc.tile_pool(name="r2p", bufs=2))
        gps = ctx.enter_context(tc.tile_pool(name="gps", bufs=3, space="PSUM"))
        csps = ctx.enter_context(tc.tile_pool(name="csps", bufs=2, space="PSUM"))

        xw = cp.tile([128, WIN], F32R, tag="xw")
        maskh = cp.tile([128, 128], F32, tag="maskh")
        maskx = cp.tile([128, 128], F32, tag="maskx")
        onesf = cp.tile([1, 128], F32, tag="onesf")
        onescolf = cp.tile([128, 1], F32, tag="onescolf")
        ones1 = cp.tile([1, 128], F32R, tag="ones1")
        onescol = cp.tile([128, 1], F32R, tag="onescol")
        sqwin = cp.tile([1, WIN], F32R, tag="sqwin")    # -sq_j/2 over window
        hbt = cp.tile([128, NT], F32R, tag="hbt")
        hb = cp.tile([128, NT], F32, tag="hb")
        zeros = cp.tile([128, 1024], F32, tag="zeros")
        hacc = cp.tile([128, NT * NMC], F32, tag="hacc")
        s32a = cp.tile([128, NT], F32, tag="s32a")
        kacc = cp.tile([128, NT], F32, tag="kacc")
        macc = cp.tile([128, NT], F32, tag="macc")

        nc.vector.memset(onesf[:], 1.0)
        nc.vector.memset(onescolf[:], -0.5)
        nc.vector.memset(zeros[:], 0.0)
        nc.vector.tensor_copy(ones1[:], onesf[:])
        nc.vector.tensor_copy(onescol[:], onescolf[:])

        for c in range(NW512):
            eng = nc.sync if c % 2 == 0 else nc.gpsimd
            eng.dma_start(xw[:, c * 512:(c + 1) * 512], xw_in[:, c * 512:(c + 1) * 512])
        nc.gpsimd.dma_start(maskh[:], maskh_in)
        nc.gpsimd.dma_start(maskx[:], maskx_in)

        # ---- interleaved emission: prep chunks appear just before the first
        # work item that needs them, so no engine stream head-of-line blocks
        # on a late DMA ----
        def emit_prep(c):
            lo = c * 512
            wc = wp.tile([128, 512], F32R, tag="wc")
            nc.vector.tensor_mul(wc[:], xw[:, lo:lo + 512], xw[:, lo:lo + 512])
            cs = csps.tile([1, 512], F32, tag="cs")
            nc.tensor.matmul(cs[:], onescol[:], wc[:], start=True, stop=True)
            if c % 2 == 0:
                nc.scalar.activation(sqwin[0:1, lo:lo + 512], cs[:], Copy)
            else:
                nc.vector.tensor_copy(sqwin[0:1, lo:lo + 512], cs[:])
            if c == 1:
                # per-anchor-tile bias (1 - sq_i)/2 via DRAM bounce;
                # anchors are window blocks 0..7 = first 1024 cols of sqwin
                scr = dp.tile([1, RPC], F32R, tag="scr")
                nc.sync.dma_start(scr[:], sqwin[0:1, 0:RPC])
                nc.sync.dma_start(hbt[:], scr[0:1, :].rearrange("o (t p) -> (o p) t", p=128))
                nc.vector.tensor_scalar(hb[:], hbt[:], 0.5, None, ADD)

        def emit_main(t, mc):
            # main unit: tile t, window columns [128(t+1)+1024mc, +1024)
            g = gps.tile([128, 1024], F32, tag="g")
            for h in range(2):
                lo = (t + 1) * 128 + mc * 1024 + h * 512
                nc.tensor.matmul(g[:, h * 512:(h + 1) * 512],
                                 xw[:, t * 128:(t + 1) * 128],
                                 xw[:, lo:lo + 512], start=True, stop=False)
                nc.tensor.matmul(g[:, h * 512:(h + 1) * 512], ones1[:],
                                 sqwin[0:1, lo:lo + 512], start=False, stop=True)
            ro = rp.tile([128, 1024], F32, tag="ro")
            s = mc * NT + t
            if t in ACT_TILES:
                nc.scalar.activation(ro[:], g[:], Relu, bias=hb[:, t:t + 1],
                                     scale=1.0, accum_out=hacc[:, s:s + 1])
            else:
                nc.vector.scalar_tensor_tensor(ro[:], g[:], hb[:, t:t + 1],
                                               zeros[:], ADD, MAX,
                                               accum_out=hacc[:, s:s + 1])

        def emit_corr(t):
            # correction: [diag block t | distance-32 block t+32] as one
            # strided N=256 matmul pair via a step-sliced AP
            g2 = csps.tile([128, 256], F32, tag="cs")
            xv = xw[:, t * 128:t * 128 + 4224].rearrange("p (c x) -> p c x", x=128)[:, ::32, :]
            sv = sqwin[0:1, t * 128:t * 128 + 4224].rearrange("o (c x) -> o c x", x=128)[:, ::32, :]
            g2v = g2[:].rearrange("p (c x) -> p c x", x=128)
            nc.tensor.matmul(g2v, xw[:, t * 128:(t + 1) * 128], xv, start=True, stop=False)
            nc.tensor.matmul(g2v, ones1[:], sv, start=False, stop=True)
            # s32: relu sum over the distance-32 half, bitwise-matching the
            # main loop's engine for this tile
            r3 = r2p.tile([128, 128], F32, tag="r3")
            if t in ACT_TILES:
                nc.scalar.activation(r3[:], g2[:, 128:256], Relu, bias=hb[:, t:t + 1],
                                     scale=1.0, accum_out=s32a[:, t:t + 1])
            else:
                nc.vector.scalar_tensor_tensor(r3[:], g2[:, 128:256], hb[:, t:t + 1],
                                               zeros[:, 0:128], ADD, MAX,
                                               accum_out=s32a[:, t:t + 1])
            # within-block cross-group hinge + homo masked sum on the diag half
            r2 = r2p.tile([128, 128], F32, tag="r2")
            nc.vector.scalar_tensor_tensor(r2[:], g2[:, 0:128], hb[:, t:t + 1],
                                           zeros[:, 0:128], ADD, MAX)
            tmp = r2p.tile([128, 128], F32, tag="tmp")
            nc.vector.scalar_tensor_tensor(tmp[:], r2[:], 0.0, maskx[:], ADD, MULT,
                                           accum_out=kacc[:, t:t + 1])
            tmp2 = r2p.tile([128, 128], F32, tag="tmp2")
            nc.vector.scalar_tensor_tensor(tmp2[:], g2[:, 0:128], 0.0, maskh[:], ADD,
                                           MULT, accum_out=macc[:, t:t + 1])

        # corrections first (their pointwise chains fill the engines while the
        # main loop's first matmuls run), then main units in column order
        items = [(t * 128 + 4224, "corr", t, 0) for t in range(NT)]
        items += [((t + 1) * 128 + (mc + 1) * 1024, "main", t, mc)
                  for mc in range(NMC) for t in range(NT)]
        c_done = 0
        for endcol, kind, t, mc in items:
            need = (endcol + 511) // 512
            while c_done < need:
                emit_prep(c_done)
                c_done += 1
            if kind == "main":
                emit_main(t, mc)
            else:
                emit_corr(t)
        while c_done < NW512:
            emit_prep(c_done)
            c_done += 1

        nc.sync.dma_start(hacc_out, hacc[:])
        nc.sync.dma_start(s32_out, s32a[:])
        nc.sync.dma_start(kacc_out, kacc[:])
        nc.sync.dma_start(macc_out, macc[:])
        nc.sync.dma_start(hb_out, hb[:])

    nc.compile()
    return nc


def kernel(x: np.ndarray):
    x = np.asarray(x, dtype=np.float32)
    assert x.shape == (B, D)

    if "nc" not in _CACHE:
        _CACHE["nc"] = _build_program()
    nc = _CACHE["nc"]

    xt = np.ascontiguousarray(x.T)  # [128, 8192]

    ii = np.arange(128)
    same = (ii[:, None] // K) == (ii[None, :] // K)
    maskh = (same & ~np.eye(128, dtype=bool)).astype(np.float32)  # same group, i!=j
    maskx = (~same).astype(np.float32)                            # cross group in-block

    in_maps = []
    for p in range(NCORES):
        cols = (np.arange(WIN) + p * RPC) % B
        in_maps.append({
            "xw_in": np.ascontiguousarray(xt[:, cols]),
            "maskh_in": maskh,
            "maskx_in": maskx,
        })

    res = bass_utils.run_bass_kernel_spmd(nc, in_maps, core_ids=list(range(NCORES)))

    raw = 0.0
    s32 = 0.0
    kcc = 0.0
    macc_tot = 0.0
    s1 = 0.0
    for p in range(NCORES):
        r = res.results[p]
        raw += r["hacc_out"].astype(np.float64).sum()
        s32 += r["s32_out"].astype(np.float64).sum()
        kcc += r["kacc_out"].astype(np.float64).sum()
        macc_tot += r["macc_out"].astype(np.float64).sum()
        a = r["hb_out"].astype(np.float64) - 0.5   # a = -sq/2 (exact)
        s1 += (-2.0 * a).sum()

    # accumulated values are relu((1-d2)/2) = relu(1-d2)/2.
    # raw covers block distances 1..31 once and distance 32 in both
    # orientations; s32 re-computes exactly those distance-32 terms (both
    # orientations, bitwise-identical), so raw - s32/2 covers every
    # cross-block unordered pair once.  kcc covers each within-block
    # cross-group pair twice.  heter_sum (one relu(1-d2) term per unordered
    # pair) = 2*(raw - s32/2) + kcc.
    heter_sum = 2.0 * raw - s32 + kcc
    homo_sum = 3.0 * s1 - 2.0 * macc_tot
    loss_homo = np.float32(homo_sum / CNT_HOMO)
    loss_heter = np.float32(heter_sum / CNT_HETER)
    return loss_homo, loss_heter



# revision 2
# speedup vs baseline: 1.5512x; 1.5512x over previous
"""Trainium2 Bass kernel for nn_MetricLoss (pairwise metric loss, B=8192 D=128 k=4).

  d2[i,j] = sq_i + sq_j - 2*x_i.x_j
  loss_homo  = sum_{same group, i!=j} d2 / 24576
  loss_heter = sum_{g_i < g_j} relu(1 - d2) / 33538048

Circular half-window sharding over 8 NeuronCores: the 8192 rows form 64
blocks of 128.  Core p owns anchor blocks 8p..8p+7.  Anchor tile t
processes column blocks t+1..t+31 (distance 1..31, each cross-block
unordered pair exactly once) plus an additive distance-32 pass (each
dist-32 pair twice globally, so it enters the heter sum at half weight).

All per-pair affine terms ride inside a single fp8e4 DoubleRow matmul
(0.5 cycles/row) by packing extra contraction rows:
  rows 0..63   x-halves (two k-tiles = 128 data dims)
  row 64       (b_n, 1) moving x (1, c_m) stationary -> + b_n + c_m
  rows 65..96  -128 * same-local-group rank-1 indicators (corr pass only)
so PSUM = G + (1 - sq_n)/2 - sq_m/2 = (1 - d2)/2 and the pointwise pass
is a bare relu+accumulate, split across ScalarE and VectorE (GPSIMD
cannot touch PSUM on TRN2).

Homo loss via the diagonal Gram blocks: homo_sum = sum (6I - 2*maskh) . G
over the 64 diag blocks (= 6*sum sq - 2*sum_maskh G), one masked
multiply-accumulate per core.
"""
import sys

sys.path.insert(0, "/opt/trn_rl_repo")

import numpy as np
import ml_dtypes
import concourse.bacc as bacc
import concourse.tile as tile
import concourse.mybir as mybir
from concourse import bass_utils
from contextlib import ExitStack

F32 = mybir.dt.float32
FP8 = mybir.dt.float8e4
DR = mybir.MatmulPerfMode.DoubleRow
F8NP = ml_dtypes.float8_e4m3

B, D, K = 8192, 128, 4
NCORES = 8
RPC = B // NCORES          # rows per core (1024)
NT = RPC // 128            # anchor tiles per core (8)
WINB = 40                  # window blocks: global blocks 8p .. 8p+39
WIN = WINB * 128           # 5120 columns
BIG = 128.0                # same-group mask magnitude (exact in fp8)
CNT_HOMO = float((B // K) * K * (K - 1))                 # 24576
CNT_HETER = float(K * K * (B // K) * (B // K - 1) // 2)  # 33538048

# accum slots in the output tile: main units (t, mc) -> 4*t + mc,
# then kacc / s32 / hom
SLOT_K = 32
SLOT_S = 33
SLOT_M = 34
NSLOT = 36

_CACHE = {}


def _build_program():
    nc = bacc.Bacc("TRN2", target_bir_lowering=False, debug=False)

    xw_in = nc.dram_tensor("xw_in", [97, 2 * WIN], FP8, kind="ExternalInput").ap()
    xs_in = nc.dram_tensor("xs_in", [97, 2 * RPC], FP8, kind="ExternalInput").ap()
    mm_in = nc.dram_tensor("mm_in", [128, 1024], F32, kind="ExternalInput").ap()
    acc_out = nc.dram_tensor("acc_out", [128, NSLOT], F32, kind="ExternalOutput").ap()

    Relu = mybir.ActivationFunctionType.Relu
    ADD = mybir.AluOpType.add
    MULT = mybir.AluOpType.mult
    MAX = mybir.AluOpType.max

    xw_r = xw_in.rearrange("p (two n) -> p two n", two=2)
    xs_r = xs_in.rearrange("p (two n) -> p two n", two=2)

    NC512 = WIN // 512  # 10 DMA chunks

    with tile.TileContext(nc) as tc, ExitStack() as ctx:
        cp = ctx.enter_context(tc.tile_pool(name="cp", bufs=1))
        sa = ctx.enter_context(tc.tile_pool(name="sa", bufs=2))
        sv = ctx.enter_context(tc.tile_pool(name="sv", bufs=2))
        pa = ctx.enter_context(tc.tile_pool(name="pa", bufs=2, space="PSUM"))
        pv = ctx.enter_context(tc.tile_pool(name="pv", bufs=2, space="PSUM"))

        xw = cp.tile([97, 2, WIN], FP8, tag="xw")
        xs = cp.tile([97, 2, RPC], FP8, tag="xs")
        maskm = cp.tile([128, 1024], F32, tag="maskm")
        zeros = cp.tile([128, 1024], F32, tag="zeros")
        acc = cp.tile([128, NSLOT], F32, tag="acc")

        nc.vector.memset(zeros[:], 0.0)
        nc.vector.memset(acc[:], 0.0)
        nc.gpsimd.dma_start(xs[:], xs_r)
        nc.gpsimd.dma_start(maskm[:], mm_in)

        def emit_prep(c):
            lo = c * 512
            eng = nc.sync if c % 2 == 0 else nc.gpsimd
            eng.dma_start(xw[:, :, lo:lo + 512], xw_r[:, :, lo:lo + 512])

        def emit_main(t, mc, on_act):
            # columns: window cols (t+1)*128 + mc*1024 .. +1024 (mc=3: +896)
            base = (t + 1) * 128 + mc * 1024
            width = 896 if mc == 3 else 1024
            pool = pa if on_act else pv
            g = pool.tile([128, 1024], F32, tag="ga" if on_act else "gv")
            st = xs[0:65, :, t * 128:(t + 1) * 128]
            for lo in range(0, width, 512):
                w = min(512, width - lo)
                nc.tensor.matmul(g[:, lo:lo + w], st,
                                 xw[0:65, :, base + lo:base + lo + w],
                                 start=True, stop=True, perf_mode=DR)
            s = 4 * t + mc
            if on_act:
                ro = sa.tile([128, 1024], F32, tag="ra")
                nc.scalar.activation(ro[:, 0:width], g[:, 0:width], Relu,
                                     bias=0.0, scale=1.0,
                                     accum_out=acc[:, s:s + 1])
            else:
                rv = sv.tile([128, 1024], F32, tag="rv")
                nc.vector.scalar_tensor_tensor(rv[:, 0:width], g[:, 0:width],
                                               0.0, zeros[:, 0:width], ADD, MAX,
                                               accum_out=acc[:, s:s + 1])

        def emit_corr(kind):
            # batched over all 8 anchor tiles: one [128, 1024] PSUM tile,
            # 8 x 128-col matmuls; start/stop only at 512-col (bank) edges
            on_act = kind in ("H", "S")
            pool = pa if on_act else pv
            g = pool.tile([128, 1024], F32, tag="ga" if on_act else "gv")
            for t in range(NT):
                if kind == "H":        # in-block hinge, group-masked
                    st = xs[0:97, :, t * 128:(t + 1) * 128]
                    mv = xw[0:97, :, t * 128:(t + 1) * 128]
                elif kind == "S":      # distance-32 blocks
                    st = xs[0:65, :, t * 128:(t + 1) * 128]
                    mv = xw[0:65, :, (t + 32) * 128:(t + 33) * 128]
                else:                  # "M": pure Gram diag for homo
                    st = xs[0:64, :, t * 128:(t + 1) * 128]
                    mv = xw[0:64, :, t * 128:(t + 1) * 128]
                nc.tensor.matmul(g[:, t * 128:(t + 1) * 128], st, mv,
                                 start=(t % 4 == 0), stop=(t % 4 == 3),
                                 perf_mode=DR, skip_group_check=True)
            if kind == "H":
                ro = sa.tile([128, 1024], F32, tag="ra")
                nc.scalar.activation(ro[:], g[:], Relu, bias=0.0, scale=1.0,
                                     accum_out=acc[:, SLOT_K:SLOT_K + 1])
            elif kind == "S":
                ro = sa.tile([128, 1024], F32, tag="ra")
                nc.scalar.activation(ro[:], g[:], Relu, bias=0.0, scale=1.0,
                                     accum_out=acc[:, SLOT_S:SLOT_S + 1])
            else:
                rv = sv.tile([128, 1024], F32, tag="rv")
                nc.vector.scalar_tensor_tensor(rv[:], g[:], 0.0, maskm[:],
                                               ADD, MULT,
                                               accum_out=acc[:, SLOT_M:SLOT_M + 1])

        # items: (needed window cols, kind, t, mc)
        items = [(1024, "corrM", 0, 0), (1024, "corrH", 0, 0)]
        for t in range(NT):
            for mc in range(4):
                width = 896 if mc == 3 else 1024
                items.append(((t + 1) * 128 + mc * 1024 + width, "main", t, mc))
        items.append((WIN, "corrS", 0, 0))
        items.sort(key=lambda it: it[0])

        c_done = 0
        n_main = 0
        for need, kind, t, mc in items:
            while c_done * 512 < need:
                emit_prep(c_done)
                c_done += 1
            if kind == "main":
                emit_main(t, mc, on_act=(n_main % 2 == 0))
                n_main += 1
            else:
                emit_corr(kind[-1])
        while c_done < NC512:
            emit_prep(c_done)
            c_done += 1

        nc.sync.dma_start(acc_out, acc[:])

    nc.compile()
    return nc


def _stage_inputs(x: np.ndarray):
    xt = np.ascontiguousarray(x.T)                      # [128, 8192] f32
    sq = (x.astype(np.float64) ** 2).sum(1)
    b = ((1.0 - sq) / 2.0).astype(np.float32)           # column bias
    c = (-sq / 2.0).astype(np.float32)                  # anchor bias

    lm = np.arange(128) // 4                            # local group ids
    gsel = (lm[None, :] == np.arange(32)[:, None])      # [32, 128] indicators

    # maskM = 6I - 2*maskh tiled 8x -> [128, 1024] f32
    same = lm[:, None] == lm[None, :]
    mM = (6.0 * np.eye(128) - 2.0 * (same & ~np.eye(128, dtype=bool))).astype(np.float32)
    mm_tile = np.ascontiguousarray(np.tile(mM, (1, 8)))

    in_maps = []
    for p in range(NCORES):
        cols = (np.arange(WIN) + p * RPC) % B
        xw8 = np.zeros((97, 2, WIN), dtype=F8NP)
        xw8[0:64, 0, :] = xt[0:64, cols].astype(F8NP)
        xw8[0:64, 1, :] = xt[64:128, cols].astype(F8NP)
        xw8[64, 0, :] = b[cols].astype(F8NP)
        xw8[64, 1, :] = np.float32(1.0)
        xw8[65:97, 0, :] = (-BIG * gsel[:, np.arange(WIN) % 128]).astype(F8NP)

        acols = cols[0:RPC]
        xs8 = np.zeros((97, 2, RPC), dtype=F8NP)
        xs8[0:64, 0, :] = xt[0:64, acols].astype(F8NP)
        xs8[0:64, 1, :] = xt[64:128, acols].astype(F8NP)
        xs8[64, 0, :] = np.float32(1.0)
        xs8[64, 1, :] = c[acols].astype(F8NP)
        xs8[65:97, 0, :] = gsel[:, np.arange(RPC) % 128].astype(F8NP)

        in_maps.append({
            "xw_in": np.ascontiguousarray(xw8.reshape(97, 2 * WIN)),
            "xs_in": np.ascontiguousarray(xs8.reshape(97, 2 * RPC)),
            "mm_in": mm_tile,
        })
    return in_maps


def kernel(x: np.ndarray):
    x = np.asarray(x, dtype=np.float32)
    assert x.shape == (B, D)

    if "nc" not in _CACHE:
        _CACHE["nc"] = _build_program()
    nc = _CACHE["nc"]

    in_maps = _stage_inputs(x)
    res = bass_utils.run_bass_kernel_spmd(nc, in_maps, core_ids=list(range(NCORES)))

    raw = 0.0
    kcc = 0.0
    s32 = 0.0
    hom = 0.0
    for p in range(NCORES):
        a = res.results[p]["acc_out"].astype(np.float64)
        raw += a[:, 0:32].sum()
        kcc += a[:, SLOT_K].sum()
        s32 += a[:, SLOT_S].sum()
        hom += a[:, SLOT_M].sum()

    # main units cover each cross-block unordered pair at distance 1..31 once;
    # the additive dist-32 pass covers those pairs twice; kcc covers each
    # in-block cross-group pair twice.  accumulated values are relu((1-d2)/2)
    # = relu(1-d2)/2.
    heter_sum = 2.0 * raw + s32 + kcc
    loss_homo = np.float32(hom / CNT_HOMO)
    loss_heter = np.float32(heter_sum / CNT_HETER)
    return loss_homo, loss_heter


# revision 6
# speedup vs baseline: 1.6007x; 1.0319x over previous
"""Trainium2 Bass kernel for nn_MetricLoss (pairwise metric loss, B=8192 D=128 k=4).

  d2[i,j] = sq_i + sq_j - 2*x_i.x_j
  loss_homo  = sum_{same group, i!=j} d2 / 24576
  loss_heter = sum_{g_i < g_j} relu(1 - d2) / 33538048

Circular half-window sharding over 8 NeuronCores: the 8192 rows form 64
blocks of 128.  Core p owns anchor blocks 8p..8p+7.  Anchor tile t
processes column blocks t+1..t+31 (distance 1..31, each cross-block
unordered pair exactly once) plus an additive distance-32 pass (each
dist-32 pair twice globally, so it enters the heter sum at half weight).

All per-pair affine terms ride inside a single fp8e4 DoubleRow matmul
(0.5 cycles/row) by packing extra contraction rows:
  rows 0..63   x-halves (two k-tiles = 128 data dims)
  row 64       (b_n, 1) moving x (1, c_m) stationary -> + b_n + c_m
  rows 65..96  -128 * same-local-group rank-1 indicators (corr pass only)
so PSUM = G + (1 - sq_n)/2 - sq_m/2 = (1 - d2)/2 and the pointwise pass
is a bare relu+accumulate, split across ScalarE and VectorE (GPSIMD
cannot touch PSUM on TRN2).

Homo loss via the diagonal Gram blocks: homo_sum = sum (6I - 2*maskh) . G
over the 64 diag blocks (= 6*sum sq - 2*sum_maskh G), one masked
multiply-accumulate per core.
"""
import sys

sys.path.insert(0, "/opt/trn_rl_repo")

import numpy as np
import ml_dtypes
import concourse.bacc as bacc
import concourse.tile as tile
import concourse.mybir as mybir
from concourse import bass_utils
from contextlib import ExitStack

F32 = mybir.dt.float32
FP8 = mybir.dt.float8e4
DR = mybir.MatmulPerfMode.DoubleRow
F8NP = ml_dtypes.float8_e4m3

B, D, K = 8192, 128, 4
NCORES = 8
RPC = B // NCORES          # rows per core (1024)
NT = RPC // 128            # anchor tiles per core (8)
WINB = 40                  # window blocks: global blocks 8p .. 8p+39
WIN = WINB * 128           # 5120 columns
BIG = 128.0                # same-group mask magnitude (exact in fp8)
CNT_HOMO = float((B // K) * K * (K - 1))                 # 24576
CNT_HETER = float(K * K * (B // K) * (B // K - 1) // 2)  # 33538048

# accum slots in the output tile: main units (t, mc) -> 4*t + mc,
# then kacc / s32 / hom
SLOT_K = 32
SLOT_S = 33
SLOT_M = 34
NSLOT = 35

_CACHE = {}


def _build_program():
    nc = bacc.Bacc("TRN2", target_bir_lowering=False, debug=False)

    xw_in = nc.dram_tensor("xw_in", [97, 2 * WIN], FP8, kind="ExternalInput").ap()
    xs_in = nc.dram_tensor("xs_in", [97, 2 * RPC], FP8, kind="ExternalInput").ap()
    mm_in = nc.dram_tensor("mm_in", [128, 1024], F32, kind="ExternalInput").ap()
    acc_out = nc.dram_tensor("acc_out", [128, NSLOT], F32, kind="ExternalOutput").ap()

    Relu = mybir.ActivationFunctionType.Relu
    ADD = mybir.AluOpType.add
    MULT = mybir.AluOpType.mult
    MAX = mybir.AluOpType.max

    xw_r = xw_in.rearrange("p (two n) -> p two n", two=2)
    xs_r = xs_in.rearrange("p (two n) -> p two n", two=2)

    NC512 = WIN // 512  # 10 DMA chunks

    with tile.TileContext(nc) as tc, ExitStack() as ctx:
        cp = ctx.enter_context(tc.tile_pool(name="cp", bufs=1))
        sa = ctx.enter_context(tc.tile_pool(name="sa", bufs=2))
        sv = ctx.enter_context(tc.tile_pool(name="sv", bufs=2))
        pa = ctx.enter_context(tc.tile_pool(name="pa", bufs=2, space="PSUM"))
        pv = ctx.enter_context(tc.tile_pool(name="pv", bufs=2, space="PSUM"))

        xw = cp.tile([97, 2, WIN], FP8, tag="xw")
        xs = cp.tile([97, 2, RPC], FP8, tag="xs")
        maskm = cp.tile([128, 1024], F32, tag="maskm")
        acc = cp.tile([128, NSLOT], F32, tag="acc")

        # xs first on the fast HWDGE (sync) queue: every matmul needs it
        nc.sync.dma_start(xs[:], xs_r)
        nc.gpsimd.dma_start(maskm[:], mm_in)

        def emit_prep(c):
            lo = c * 512
            eng = nc.sync if c % 2 == 0 else nc.gpsimd
            eng.dma_start(xw[:, :, lo:lo + 512], xw_r[:, :, lo:lo + 512])

        def emit_main(t, mc, on_act):
            # columns: window cols (t+1)*128 + mc*1024 .. +1024 (mc=3: +896)
            base = (t + 1) * 128 + mc * 1024
            width = 896 if mc == 3 else 1024
            pool = pa if on_act else pv
            g = pool.tile([128, 1024], F32, tag="ga" if on_act else "gv")
            st = xs[0:65, :, t * 128:(t + 1) * 128]
            for lo in range(0, width, 512):
                w = min(512, width - lo)
                nc.tensor.matmul(g[:, lo:lo + w], st,
                                 xw[0:65, :, base + lo:base + lo + w],
                                 start=True, stop=True, perf_mode=DR)
            s = 4 * t + mc
            if on_act:
                ro = sa.tile([128, 1024], F32, tag="ra")
                nc.scalar.activation(ro[:, 0:width], g[:, 0:width], Relu,
                                     bias=0.0, scale=1.0,
                                     accum_out=acc[:, s:s + 1])
            else:
                rv = sv.tile([128, 1024], F32, tag="rv")
                nc.vector.tensor_scalar(rv[:, 0:width], g[:, 0:width],
                                        0.0, 0.0, MAX, ADD,
                                        accum_out=acc[:, s:s + 1])

        def emit_corr(kind):
            # batched over all 8 anchor tiles: one [128, 1024] PSUM tile,
            # 8 x 128-col matmuls; start/stop only at 512-col (bank) edges
            on_act = kind in ("H", "S")
            pool = pa if on_act else pv
            g = pool.tile([128, 1024], F32, tag="ga" if on_act else "gv")
            for t in range(NT):
                if kind == "H":        # in-block hinge, group-masked
                    st = xs[0:97, :, t * 128:(t + 1) * 128]
                    mv = xw[0:97, :, t * 128:(t + 1) * 128]
                elif kind == "S":      # distance-32 blocks
                    st = xs[0:65, :, t * 128:(t + 1) * 128]
                    mv = xw[0:65, :, (t + 32) * 128:(t + 33) * 128]
                else:                  # "M": pure Gram diag for homo
                    st = xs[0:64, :, t * 128:(t + 1) * 128]
                    mv = xw[0:64, :, t * 128:(t + 1) * 128]
                nc.tensor.matmul(g[:, t * 128:(t + 1) * 128], st, mv,
                                 start=(t % 4 == 0), stop=(t % 4 == 3),
                                 perf_mode=DR, skip_group_check=True)
            if kind == "H":
                ro = sa.tile([128, 1024], F32, tag="ra")
                nc.scalar.activation(ro[:], g[:], Relu, bias=0.0, scale=1.0,
                                     accum_out=acc[:, SLOT_K:SLOT_K + 1])
            elif kind == "S":
                ro = sa.tile([128, 1024], F32, tag="ra")
                nc.scalar.activation(ro[:], g[:], Relu, bias=0.0, scale=1.0,
                                     accum_out=acc[:, SLOT_S:SLOT_S + 1])
            else:
                rv = sv.tile([128, 1024], F32, tag="rv")
                nc.vector.scalar_tensor_tensor(rv[:], g[:], 0.0, maskm[:],
                                               ADD, MULT,
                                               accum_out=acc[:, SLOT_M:SLOT_M + 1])

        # items: (needed window cols, kind, t, mc)
        items = [(1024, "corrM", 0, 0), (1024, "corrH", 0, 0)]
        for t in range(NT):
            for mc in range(4):
                width = 896 if mc == 3 else 1024
                items.append(((t + 1) * 128 + mc * 1024 + width, "main", t, mc))
        items.append((WIN, "corrS", 0, 0))
        items.sort(key=lambda it: it[0])

        # engine split: Act gets 15 mains + corrH + corrS (17 units),
        # DVE gets 17 mains + corrM (18 units)
        c_done = 0
        n_main = 0
        for need, kind, t, mc in items:
            while c_done * 512 < need:
                emit_prep(c_done)
                c_done += 1
            if kind == "main":
                on_act = (n_main % 2 == 1) and n_main not in (1, 17)
                emit_main(t, mc, on_act=on_act)
                n_main += 1
            else:
                emit_corr(kind[-1])
        while c_done < NC512:
            emit_prep(c_done)
            c_done += 1

        nc.sync.dma_start(acc_out, acc[:])

    nc.compile()
    return nc


def _stage_inputs(x: np.ndarray):
    xt = np.ascontiguousarray(x.T)                      # [128, 8192] f32
    sq = (x.astype(np.float64) ** 2).sum(1)
    b = ((1.0 - sq) / 2.0).astype(np.float32)           # column bias
    c = (-sq / 2.0).astype(np.float32)                  # anchor bias

    lm = np.arange(128) // 4                            # local group ids
    gsel = (lm[None, :] == np.arange(32)[:, None])      # [32, 128] indicators

    # maskM = 6I - 2*maskh tiled 8x -> [128, 1024] f32
    same = lm[:, None] == lm[None, :]
    mM = (6.0 * np.eye(128) - 2.0 * (same & ~np.eye(128, dtype=bool))).astype(np.float32)
    mm_tile = np.ascontiguousarray(np.tile(mM, (1, 8)))

    in_maps = []
    for p in range(NCORES):
        cols = (np.arange(WIN) + p * RPC) % B
        xw8 = np.zeros((97, 2, WIN), dtype=F8NP)
        xw8[0:64, 0, :] = xt[0:64, cols].astype(F8NP)
        xw8[0:64, 1, :] = xt[64:128, cols].astype(F8NP)
        xw8[64, 0, :] = b[cols].astype(F8NP)
        xw8[64, 1, :] = np.float32(1.0)
        xw8[65:97, 0, :] = (-BIG * gsel[:, np.arange(WIN) % 128]).astype(F8NP)

        acols = cols[0:RPC]
        xs8 = np.zeros((97, 2, RPC), dtype=F8NP)
        xs8[0:64, 0, :] = xt[0:64, acols].astype(F8NP)
        xs8[0:64, 1, :] = xt[64:128, acols].astype(F8NP)
        xs8[64, 0, :] = np.float32(1.0)
        xs8[64, 1, :] = c[acols].astype(F8NP)
        xs8[65:97, 0, :] = gsel[:, np.arange(RPC) % 128].astype(F8NP)

        in_maps.append({
            "xw_in": np.ascontiguousarray(xw8.reshape(97, 2 * WIN)),
            "xs_in": np.ascontiguousarray(xs8.reshape(97, 2 * RPC)),
            "mm_in": mm_tile,
        })
    return in_maps


def kernel(x: np.ndarray):
    x = np.asarray(x, dtype=np.float32)
    assert x.shape == (B, D)

    if "nc" not in _CACHE:
        _CACHE["nc"] = _build_program()
    nc = _CACHE["nc"]

    in_maps = _stage_inputs(x)
    res = bass_utils.run_bass_kernel_spmd(nc, in_maps, core_ids=list(range(NCORES)))

    raw = 0.0
    kcc = 0.0
    s32 = 0.0
    hom = 0.0
    for p in range(NCORES):
        a = res.results[p]["acc_out"].astype(np.float64)
        raw += a[:, 0:32].sum()
        kcc += a[:, SLOT_K].sum()
        s32 += a[:, SLOT_S].sum()
        hom += a[:, SLOT_M].sum()

    # main units cover each cross-block unordered pair at distance 1..31 once;
    # the additive dist-32 pass covers those pairs twice; kcc covers each
    # in-block cross-group pair twice.  accumulated values are relu((1-d2)/2)
    # = relu(1-d2)/2.
    heter_sum = 2.0 * raw + s32 + kcc
    loss_homo = np.float32(hom / CNT_HOMO)
    loss_heter = np.float32(heter_sum / CNT_HETER)
    return loss_homo, loss_heter


# revision 12
# speedup vs baseline: 1.6866x; 1.0536x over previous
"""Trainium2 Bass kernel for nn_MetricLoss (pairwise metric loss, B=8192 D=128 k=4).

  d2[i,j] = sq_i + sq_j - 2*x_i.x_j
  loss_homo  = sum_{same group, i!=j} d2 / 24576
  loss_heter = sum_{g_i < g_j} relu(1 - d2) / 33538048

Circular half-window sharding over 8 NeuronCores: the 8192 rows form 64
blocks of 128.  Core p owns anchor blocks 8p..8p+7.  Anchor tile t
processes column blocks t+1..t+31 (distance 1..31, each cross-block
unordered pair exactly once) plus an additive distance-32 pass (each
dist-32 pair twice globally, so it enters the heter sum at half weight).

All per-pair affine terms ride inside a single fp8e4 DoubleRow matmul
(0.5 cycles/row) by packing extra contraction rows:
  rows 0..63   x-halves (two k-tiles = 128 data dims)
  row 64       (b_n, 1) moving x (1, c_m) stationary -> + b_n + c_m
  rows 65..96  -128 * same-local-group rank-1 indicators (corr pass only)
so PSUM = G + (1 - sq_n)/2 - sq_m/2 = (1 - d2)/2 and the pointwise pass
is a bare relu+accumulate, split across ScalarE and VectorE (GPSIMD
cannot touch PSUM on TRN2).

Homo loss via the diagonal Gram blocks: homo_sum = sum (6I - 2*maskh) . G
over the 64 diag blocks (= 6*sum sq - 2*sum_maskh G), one masked
multiply-accumulate per core.
"""
import sys

sys.path.insert(0, "/opt/trn_rl_repo")

import numpy as np
import ml_dtypes
import concourse.bacc as bacc
import concourse.tile as tile
import concourse.mybir as mybir
from concourse import bass_utils
from contextlib import ExitStack

F32 = mybir.dt.float32
FP8 = mybir.dt.float8e4
DR = mybir.MatmulPerfMode.DoubleRow
F8NP = ml_dtypes.float8_e4m3

B, D, K = 8192, 128, 4
NCORES = 8
RPC = B // NCORES          # rows per core (1024)
NT = RPC // 128            # anchor tiles per core (8)
WINB = 40                  # window blocks: global blocks 8p .. 8p+39
WIN = WINB * 128           # 5120 columns
BIG = 128.0                # same-group mask magnitude (exact in fp8)
CNT_HOMO = float((B // K) * K * (K - 1))                 # 24576
CNT_HETER = float(K * K * (B // K) * (B // K - 1) // 2)  # 33538048

# accum slots in the output tile: main units (t, mc) -> 4*t + mc,
# then kacc / s32 / hom
SLOT_K = 32
SLOT_S = 33
SLOT_M = 34
NSLOT = 35

_CACHE = {}


def _build_program():
    nc = bacc.Bacc("TRN2", target_bir_lowering=False, debug=False)

    xw_in = nc.dram_tensor("xw_in", [97, 2 * WIN], FP8, kind="ExternalInput").ap()
    xs_in = nc.dram_tensor("xs_in", [97, 2 * RPC], FP8, kind="ExternalInput").ap()
    mm_in = nc.dram_tensor("mm_in", [128, 1024], F32, kind="ExternalInput").ap()
    acc_out = nc.dram_tensor("acc_out", [128, NSLOT], F32, kind="ExternalOutput").ap()

    Relu = mybir.ActivationFunctionType.Relu
    ADD = mybir.AluOpType.add
    MULT = mybir.AluOpType.mult
    MAX = mybir.AluOpType.max

    xw_r = xw_in.rearrange("p (two n) -> p two n", two=2)
    xs_r = xs_in.rearrange("p (two n) -> p two n", two=2)

    with tile.TileContext(nc) as tc, ExitStack() as ctx:
        cp = ctx.enter_context(tc.tile_pool(name="cp", bufs=1))
        pa = ctx.enter_context(tc.tile_pool(name="pa", bufs=2, space="PSUM"))
        pv = ctx.enter_context(tc.tile_pool(name="pv", bufs=2, space="PSUM"))

        xw = cp.tile([97, 2, WIN], FP8, tag="xw")
        xs = cp.tile([97, 2, RPC], FP8, tag="xs")
        maskm = cp.tile([128, 1024], F32, tag="maskm")
        zeros = cp.tile([128, 1024], F32, tag="zeros")
        acc = cp.tile([128, NSLOT], F32, tag="acc")

        # xs first on the fast HWDGE (sync) queue: every matmul needs it.
        # maskm (f32, slow transfer) goes LAST on the gpsimd queue — its
        # consumer (corrM) is emitted mid-stream.
        nc.sync.dma_start(xs[:], xs_r)
        nc.vector.memset(zeros[:], 0.0)

        def emit_prep(c):
            # 1024-col DMA chunks, alternating queues
            lo = c * 1024
            eng = nc.sync if c % 2 == 0 else nc.gpsimd
            eng.dma_start(xw[:, :, lo:lo + 1024], xw_r[:, :, lo:lo + 1024])

        def emit_main(t, mc, on_act):
            # columns: window cols (t+1)*128 + mc*1024 .. +1024 (mc=3: +896)
            base = (t + 1) * 128 + mc * 1024
            width = 896 if mc == 3 else 1024
            pool = pa if on_act else pv
            g = pool.tile([128, 1024], F32, tag="ga" if on_act else "gv")
            st = xs[0:65, :, t * 128:(t + 1) * 128]
            for lo in range(0, width, 512):
                w = min(512, width - lo)
                nc.tensor.matmul(g[:, lo:lo + w], st,
                                 xw[0:65, :, base + lo:base + lo + w],
                                 start=True, stop=True, perf_mode=DR)
            s = 4 * t + mc
            if on_act:
                nc.scalar.activation(g[:, 0:width], g[:, 0:width], Relu,
                                     bias=0.0, scale=1.0,
                                     accum_out=acc[:, s:s + 1])
            else:
                nc.vector.scalar_tensor_tensor(g[:, 0:width], g[:, 0:width],
                                               0.0, zeros[:, 0:width], ADD, MAX,
                                               accum_out=acc[:, s:s + 1])

        def emit_corr(kind):
            # batched over all 8 anchor tiles: one [128, 1024] PSUM tile,
            # 8 x 128-col matmuls; start/stop only at 512-col (bank) edges
            on_act = kind in ("H", "S")
            pool = pa if on_act else pv
            g = pool.tile([128, 1024], F32, tag="ga" if on_act else "gv")
            for t in range(NT):
                if kind == "H":        # in-block hinge, group-masked
                    st = xs[0:97, :, t * 128:(t + 1) * 128]
                    mv = xw[0:97, :, t * 128:(t + 1) * 128]
                elif kind == "S":      # distance-32 blocks
                    st = xs[0:65, :, t * 128:(t + 1) * 128]
                    mv = xw[0:65, :, (t + 32) * 128:(t + 33) * 128]
                else:                  # "M": pure Gram diag for homo
                    st = xs[0:64, :, t * 128:(t + 1) * 128]
                    mv = xw[0:64, :, t * 128:(t + 1) * 128]
                nc.tensor.matmul(g[:, t * 128:(t + 1) * 128], st, mv,
                                 start=(t % 4 == 0), stop=(t % 4 == 3),
                                 perf_mode=DR, skip_group_check=True)
            if kind == "H":
                nc.scalar.activation(g[:], g[:], Relu, bias=0.0, scale=1.0,
                                     accum_out=acc[:, SLOT_K:SLOT_K + 1])
            elif kind == "S":
                nc.scalar.activation(g[:], g[:], Relu, bias=0.0, scale=1.0,
                                     accum_out=acc[:, SLOT_S:SLOT_S + 1])
            else:
                nc.vector.scalar_tensor_tensor(g[:], g[:], 0.0, maskm[:],
                                               ADD, MULT,
                                               accum_out=acc[:, SLOT_M:SLOT_M + 1])

        # items: (needed window cols, sort order, kind, t, mc).  corrH early
        # (first Act work), corrM after a few mains (its maskm DMA is late),
        # corrS at the end.
        items = [(1024, 0, "corrH", 0, 0), (1024, 2500, "corrM", 0, 0)]
        for t in range(NT):
            for mc in range(4):
                width = 896 if mc == 3 else 1024
                need = (t + 1) * 128 + mc * 1024 + width
                items.append((need, need, "main", t, mc))
        items.append((WIN, WIN + 1, "corrS", 0, 0))
        items.sort(key=lambda it: it[1])

        # engine split: Act gets 16 mains + corrH + corrS (18 units),
        # DVE gets 16 mains + corrM (17 units)
        c_done = 0
        n_main = 0
        for need, _, kind, t, mc in items:
            while c_done * 1024 < need:
                emit_prep(c_done)
                c_done += 1
            if kind == "main":
                emit_main(t, mc, on_act=(n_main % 2 == 0))
                n_main += 1
            else:
                if kind == "corrM":
                    nc.gpsimd.dma_start(maskm[:], mm_in)
                emit_corr(kind[-1])
        while c_done * 1024 < WIN:
            emit_prep(c_done)
            c_done += 1

        nc.sync.dma_start(acc_out, acc[:])

    nc.compile()
    return nc


def _stage_inputs(x: np.ndarray):
    xt = np.ascontiguousarray(x.T)                      # [128, 8192] f32
    sq = (x.astype(np.float64) ** 2).sum(1)
    b = ((1.0 - sq) / 2.0).astype(np.float32)           # column bias
    c = (-sq / 2.0).astype(np.float32)                  # anchor bias

    lm = np.arange(128) // 4                            # local group ids
    gsel = (lm[None, :] == np.arange(32)[:, None])      # [32, 128] indicators

    # maskM = 6I - 2*maskh tiled 8x -> [128, 1024] f32
    same = lm[:, None] == lm[None, :]
    mM = (6.0 * np.eye(128) - 2.0 * (same & ~np.eye(128, dtype=bool))).astype(np.float32)
    mm_tile = np.ascontiguousarray(np.tile(mM, (1, 8)))

    in_maps = []
    for p in range(NCORES):
        cols = (np.arange(WIN) + p * RPC) % B
        xw8 = np.zeros((97, 2, WIN), dtype=F8NP)
        xw8[0:64, 0, :] = xt[0:64, cols].astype(F8NP)
        xw8[0:64, 1, :] = xt[64:128, cols].astype(F8NP)
        xw8[64, 0, :] = b[cols].astype(F8NP)
        xw8[64, 1, :] = np.float32(1.0)
        xw8[65:97, 0, :] = (-BIG * gsel[:, np.arange(WIN) % 128]).astype(F8NP)

        acols = cols[0:RPC]
        xs8 = np.zeros((97, 2, RPC), dtype=F8NP)
        xs8[0:64, 0, :] = xt[0:64, acols].astype(F8NP)
        xs8[0:64, 1, :] = xt[64:128, acols].astype(F8NP)
        xs8[64, 0, :] = np.float32(1.0)
        xs8[64, 1, :] = c[acols].astype(F8NP)
        xs8[65:97, 0, :] = gsel[:, np.arange(RPC) % 128].astype(F8NP)

        in_maps.append({
            "xw_in": np.ascontiguousarray(xw8.reshape(97, 2 * WIN)),
            "xs_in": np.ascontiguousarray(xs8.reshape(97, 2 * RPC)),
            "mm_in": mm_tile,
        })
    return in_maps


def kernel(x: np.ndarray):
    x = np.asarray(x, dtype=np.float32)
    assert x.shape == (B, D)

    if "nc" not in _CACHE:
        _CACHE["nc"] = _build_program()
    nc = _CACHE["nc"]

    in_maps = _stage_inputs(x)
    res = bass_utils.run_bass_kernel_spmd(nc, in_maps, core_ids=list(range(NCORES)))

    raw = 0.0
    kcc = 0.0
    s32 = 0.0
    hom = 0.0
    for p in range(NCORES):
        a = res.results[p]["acc_out"].astype(np.float64)
        raw += a[:, 0:32].sum()
        kcc += a[:, SLOT_K].sum()
        s32 += a[:, SLOT_S].sum()
        hom += a[:, SLOT_M].sum()

    # main units cover each cross-block unordered pair at distance 1..31 once;
    # the additive dist-32 pass covers those pairs twice; kcc covers each
    # in-block cross-group pair twice.  accumulated values are relu((1-d2)/2)
    # = relu(1-d2)/2.
    heter_sum = 2.0 * raw + s32 + kcc
    loss_homo = np.float32(hom / CNT_HOMO)
    loss_heter = np.float32(heter_sum / CNT_HETER)
    return loss_homo, loss_heter
